# revision 5
# baseline (speedup 1.0000x reference)
"""Multi-head causal attention (GQA + QK-RMSNorm + RoPE) for Trainium2, 8 cores.

Sharding: 8 cores = 2 batches x 4 head-groups (tensor parallel over heads).
Each core handles one batch and 8 Q heads + 2 KV heads:
  - QKV projection for its head slice (fp16 matmuls, fp32 psum)
  - QK RMSNorm + RoPE (gamma folded into host-side cos/sin tables)
  - causal attention in k-major (transposed-scores) layout:
      sT[k, q] = kT.T @ qT ; p = exp(sT/8) ; ctx = pT.T @ [v | 1]
    (the ones column yields the softmax denominator for free)
  - output projection against its w_out column slice -> partial [S, D]
Host sums the 4 head-group partials per batch.
"""
import sys
import numpy as np
from contextlib import ExitStack

if '/opt/trn_rl_repo' not in sys.path:
    sys.path.insert(0, '/opt/trn_rl_repo')

import concourse.bacc as bacc
import concourse.tile as tile
import concourse.mybir as mybir
from concourse.bass_utils import run_bass_kernel_spmd

dt = mybir.dt
AF = mybir.ActivationFunctionType
AX = mybir.AxisListType
ALU = mybir.AluOpType

HEAD_DIM = 64
NUM_Q_HEADS = 32
NUM_KV_HEADS = 8
ROPE_FREQ = 10000.0
EPS = 1e-6

B, S, D = 2, 2048, 2048
QH = 8            # q heads per core
KVH = 2           # kv heads per core
N_CORES = 8
ST = S // 128      # 16 token tiles of 128
NJ = S // 512      # 4 big q blocks of 512

F16 = dt.float16
F32 = dt.float32


def _build():
    nc = bacc.Bacc("TRN2", target_bir_lowering=False, debug=False,
                   num_devices=N_CORES)

    xT = nc.dram_tensor("xT", [D, S], F16, kind="ExternalInput").ap()
    wqkvT = nc.dram_tensor("wqkvT", [D, (QH + 2 * KVH) * HEAD_DIM], F16,
                           kind="ExternalInput").ap()          # cols: 512 q | 128 k | 128 v
    woutT = nc.dram_tensor("woutT", [QH * HEAD_DIM, D], F16,
                           kind="ExternalInput").ap()
    cos_q = nc.dram_tensor("cos_q", [S, HEAD_DIM], F32, kind="ExternalInput").ap()
    sin_q = nc.dram_tensor("sin_q", [S, HEAD_DIM], F32, kind="ExternalInput").ap()
    cos_k = nc.dram_tensor("cos_k", [S, HEAD_DIM], F32, kind="ExternalInput").ap()
    sin_k = nc.dram_tensor("sin_k", [S, HEAD_DIM], F32, kind="ExternalInput").ap()
    mask_i = nc.dram_tensor("mask_i", [128, 128], F16, kind="ExternalInput").ap()
    ident_i = nc.dram_tensor("ident_i", [128, 128], F16, kind="ExternalInput").ap()
    out = nc.dram_tensor("out", [S, D], F32, kind="ExternalOutput").ap()

    with tile.TileContext(nc) as tc, ExitStack() as ctx:
        perm = ctx.enter_context(tc.tile_pool(name="perm", bufs=1))

        # ---- persistent tiles ----
        w_ch = [perm.tile([128, 768], F16, tag=f"w{dc}", name=f"w{dc}") for dc in range(16)]
        for dc in range(16):
            nc.sync.dma_start(w_ch[dc][:], wqkvT[128 * dc:128 * (dc + 1), :])
        wo_ch = [perm.tile([128, D], F16, tag=f"wo{p}", name=f"wo{p}") for p in range(4)]
        for p in range(4):
            nc.sync.dma_start(wo_ch[p][:], woutT[128 * p:128 * (p + 1), :])
        tabs = {}
        for nm, src in (("cq", cos_q), ("sq", sin_q), ("ck", cos_k), ("sk", sin_k)):
            t = perm.tile([128, ST, HEAD_DIM], F32, tag=f"tab{nm}", name=f"tab{nm}")
            nc.sync.dma_start(t[:], src.rearrange("(n p) d -> p n d", p=128))
            tabs[nm] = t
        tmask = perm.tile([128, 128], F16, tag="mask")
        nc.sync.dma_start(tmask[:], mask_i[:])
        tident = perm.tile([128, 128], F16, tag="ident")
        nc.sync.dma_start(tident[:], ident_i[:])
        teps = perm.tile([128, 1], F32, tag="eps")
        nc.vector.memset(teps[:], EPS)

        qT = [perm.tile([128, S], F16, tag=f"qT{p}", name=f"qT{p}") for p in range(4)]
        kT = [perm.tile([128, S], F16, tag=f"kT{g}", name=f"kT{g}") for g in range(KVH)]
        vext = [[perm.tile([128, HEAD_DIM + 1], F16, tag=f"v{g}_{i}", name=f"v{g}_{i}")
                 for i in range(ST)] for g in range(KVH)]
        for g in range(KVH):
            for i in range(ST):
                nc.vector.memset(vext[g][i][:, HEAD_DIM:HEAD_DIM + 1], 1.0)
        ctxT = [perm.tile([128, S], F16, tag=f"ctxT{p}", name=f"ctxT{p}") for p in range(4)]

        # ================= Phase 1: QKV + norm + rope + transposes ============
        with tc.tile_pool(name="p1sb", bufs=3) as p1sb, \
             tc.tile_pool(name="p1st", bufs=2) as p1st, \
             tc.tile_pool(name="psq", bufs=2, space="PSUM") as psq, \
             tc.tile_pool(name="pskv", bufs=2, space="PSUM") as pskv, \
             tc.tile_pool(name="pstp", bufs=2, space="PSUM") as pstp:
            for i2 in range(ST // 2):            # load x in 256-token chunks
                xt = p1sb.tile([128, 16, 256], F16, tag="xt")
                nc.sync.dma_start(
                    xt[:], xT[:, 256 * i2:256 * (i2 + 1)]
                    .rearrange("(n p) s -> p n s", p=128))
                for ii in range(2):
                    i = 2 * i2 + ii
                    pq = psq.tile([128, 512], F32, tag="pq")
                    pkv = pskv.tile([128, 256], F32, tag="pkv")
                    for dc in range(16):
                        xs = xt[:, dc, 128 * ii:128 * (ii + 1)]
                        nc.tensor.matmul(pq[:], xs, w_ch[dc][:, 0:512],
                                         start=(dc == 0), stop=(dc == 15))
                        nc.tensor.matmul(pkv[:], xs, w_ch[dc][:, 512:768],
                                         start=(dc == 0), stop=(dc == 15))
                    qsb = p1sb.tile([128, 640], F16, tag="qsb")
                    nc.vector.tensor_copy(qsb[:, 0:512], pq[:])
                    nc.vector.tensor_copy(qsb[:, 512:640], pkv[:, 0:128])
                    # v chunks straight out to vext tiles
                    nc.vector.tensor_copy(vext[0][i][:, 0:HEAD_DIM], pkv[:, 128:192])
                    nc.vector.tensor_copy(vext[1][i][:, 0:HEAD_DIM], pkv[:, 192:256])

                    # --- RMSNorm (per 64) + RoPE on q (8 heads) and k (2 heads)
                    sqt = p1sb.tile([128, 640], F16, tag="sqt")
                    nc.vector.tensor_mul(sqt[:], qsb[:], qsb[:])
                    ssum = p1st.tile([128, 10], F32, tag="ssum")
                    nc.vector.tensor_reduce(
                        ssum[:], sqt[:].rearrange("p (h d) -> p h d", h=10),
                        axis=AX.X, op=ALU.add)
                    std = p1st.tile([128, 10], F32, tag="std")
                    nc.scalar.activation(std[:], ssum[:], AF.Sqrt,
                                         bias=teps[:], scale=1.0 / HEAD_DIM)
                    rstd = p1st.tile([128, 10], F32, tag="rstd")
                    nc.vector.reciprocal(rstd[:], std[:])
                    qn = p1sb.tile([128, 640], F16, tag="qn")
                    nc.vector.tensor_mul(
                        qn[:].rearrange("p (h d) -> p h d", h=10),
                        qsb[:].rearrange("p (h d) -> p h d", h=10),
                        rstd[:].unsqueeze(2).broadcast_to([128, 10, HEAD_DIM]))
                    qr = p1sb.tile([128, 640], F16, tag="qr")
                    t2 = p1sb.tile([128, 640], F16, tag="t2")
                    for part, nh, c_t, s_t in (("q", QH, "cq", "sq"),
                                               ("k", KVH, "ck", "sk")):
                        off = 0 if part == "q" else 512
                        qn3 = qn[:, off:off + 64 * nh].rearrange(
                            "p (h d) -> p h d", h=nh)
                        qn4 = qn[:, off:off + 64 * nh].rearrange(
                            "p (h two x) -> p h two x", h=nh, two=2)
                        cosb = tabs[c_t][:, i, :].unsqueeze(1) \
                            .broadcast_to([128, nh, HEAD_DIM])
                        sin4 = tabs[s_t][:, i, :].unsqueeze(1) \
                            .broadcast_to([128, nh, HEAD_DIM]) \
                            .rearrange("p h (two x) -> p h two x", two=2)
                        t2v = t2[:, off:off + 64 * nh].rearrange(
                            "p (h two x) -> p h two x", h=nh, two=2)
                        nc.vector.tensor_mul(t2v[:, :, 0, :], qn4[:, :, 1, :],
                                             sin4[:, :, 0, :])
                        nc.vector.tensor_mul(t2v[:, :, 1, :], qn4[:, :, 0, :],
                                             sin4[:, :, 1, :])
                        # qr = qn*cos + t2  (two ops: mul into qr, then add)
                        qr3 = qr[:, off:off + 64 * nh].rearrange(
                            "p (h d) -> p h d", h=nh)
                        nc.vector.tensor_mul(qr3, qn3, cosb)
                    nc.vector.tensor_add(qr[:], qr[:], t2[:])

                    # --- transposes into [hd, S] pair layout
                    for p in range(4):
                        tp = pstp.tile([128, 128], F16, tag="tpq")
                        nc.tensor.transpose(tp[0:64, :], qr[:, 128 * p:128 * p + 64],
                                            tident[:])
                        nc.tensor.transpose(tp[64:128, :],
                                            qr[:, 128 * p + 64:128 * p + 128],
                                            tident[:])
                        nc.vector.tensor_copy(qT[p][:, 128 * i:128 * (i + 1)], tp[:])
                    for g in range(KVH):
                        tp = pstp.tile([128, 128], F16, tag="tpk")
                        nc.tensor.transpose(tp[0:64, :],
                                            qr[:, 512 + 64 * g:512 + 64 * (g + 1)],
                                            tident[:])
                        nc.tensor.transpose(tp[64:128, :],
                                            qr[:, 512 + 64 * g:512 + 64 * (g + 1)],
                                            tident[:])
                        nc.vector.tensor_copy(kT[g][:, 128 * i:128 * (i + 1)], tp[:])

        # ================= Phase 2: attention =================================
        with tc.tile_pool(name="pssT", bufs=2, space="PSUM") as pssT, \
             tc.tile_pool(name="psctx", bufs=1, space="PSUM") as psctx, \
             tc.tile_pool(name="pstp2", bufs=2, space="PSUM") as pstp2, \
             tc.tile_pool(name="ptpool", bufs=4) as ptpool, \
             tc.tile_pool(name="cnpool", bufs=2) as cnpool, \
             tc.tile_pool(name="rcpool", bufs=8) as rcpool:
            for J in range(NJ):
                ctxn = [cnpool.tile([128, 512], F16, tag=f"cn{jj}", name=f"cn{jj}")
                        for jj in range(4)]
                for h in range(QH):
                    g = h // 4          # kv head
                    pair = h // 2
                    half = 64 * (h % 2)
                    cps = [psctx.tile([128, HEAD_DIM + 1], F32, tag=f"ctx{jj}", name=f"ctx{jj}")
                           for jj in range(4)]
                    for c in range(4 * J + 4):
                        sT = pssT.tile([128, 512], F32, tag="sT")
                        nc.tensor.matmul(
                            sT[:], kT[g][half:half + 64, 128 * c:128 * (c + 1)],
                            qT[pair][half:half + 64, 512 * J:512 * (J + 1)],
                            start=True, stop=True)
                        jj0 = max(0, c - 4 * J)
                        pt = ptpool.tile([128, 512], F16, tag="pt")
                        nc.scalar.activation(pt[:, 128 * jj0:512],
                                             sT[:, 128 * jj0:512],
                                             AF.Exp, scale=0.125)
                        if c >= 4 * J:      # diagonal block: triangular mask
                            nc.vector.tensor_mul(
                                pt[:, 128 * jj0:128 * (jj0 + 1)],
                                pt[:, 128 * jj0:128 * (jj0 + 1)], tmask[:])
                        for jj in range(jj0, 4):
                            nc.tensor.matmul(
                                cps[jj][:], pt[:, 128 * jj:128 * (jj + 1)],
                                vext[g][c][:],
                                start=(c == 0), stop=(c == 4 * J + jj))
                    for jj in range(4):
                        rc = rcpool.tile([128, 1], F32, tag="rc")
                        nc.vector.reciprocal(
                            rc[:], cps[jj][:, HEAD_DIM:HEAD_DIM + 1])
                        nc.vector.tensor_scalar_mul(
                            ctxn[jj][:, 64 * h:64 * (h + 1)],
                            cps[jj][:, 0:HEAD_DIM], rc[:])
                for jj in range(4):
                    for p in range(4):
                        tp = pstp2.tile([128, 128], F16, tag="tp2")
                        nc.tensor.transpose(
                            tp[0:64, :], ctxn[jj][:, 128 * p:128 * p + 64],
                            tident[:])
                        nc.tensor.transpose(
                            tp[64:128, :], ctxn[jj][:, 128 * p + 64:128 * (p + 1)],
                            tident[:])
                        nc.vector.tensor_copy(
                            ctxT[p][:, 512 * J + 128 * jj:512 * J + 128 * (jj + 1)],
                            tp[:])

        # ================= Phase 3: output projection =========================
        with tc.tile_pool(name="psout", bufs=4, space="PSUM") as psout, \
             tc.tile_pool(name="osb", bufs=4) as osb:
            for i in range(ST):
                for dch in range(4):
                    po = psout.tile([128, 512], F32, tag="po")
                    for p in range(4):
                        nc.tensor.matmul(
                            po[:], ctxT[p][:, 128 * i:128 * (i + 1)],
                            wo_ch[p][:, 512 * dch:512 * (dch + 1)],
                            start=(p == 0), stop=(p == 3))
                    ob = osb.tile([128, 512], F32, tag="ob")
                    nc.scalar.copy(ob[:], po[:])
                    nc.sync.dma_start(
                        out[128 * i:128 * (i + 1), 512 * dch:512 * (dch + 1)],
                        ob[:])

    nc.compile()
    return nc


_NC = None


def _get_nc():
    global _NC
    if _NC is None:
        _NC = _build()
    return _NC


_RUNNER = None


def _get_runner():
    """Build (once) a jitted 8-core sharded callable around the bass program.

    Slim replica of bass2jax.run_bass_via_pjrt's multi-core path, kept
    reusable so repeated invocations skip retracing/recompilation.
    """
    global _RUNNER
    if _RUNNER is not None:
        return _RUNNER
    import jax
    from jax.sharding import Mesh, PartitionSpec
    from jax.experimental.shard_map import shard_map
    from concourse import bass2jax
    from concourse import mybir as _mybir

    nc = _get_nc()
    bass2jax.install_neuronx_cc_hook()

    partition_name = nc.partition_id_tensor.name if nc.partition_id_tensor else None
    in_names, out_names, out_avals, zero_outs = [], [], [], []
    for alloc in nc.m.functions[0].allocations:
        if not isinstance(alloc, _mybir.MemoryLocationSet):
            continue
        name = alloc.memorylocations[0].name
        if alloc.kind == "ExternalInput":
            if name != partition_name:
                in_names.append(name)
        elif alloc.kind == "ExternalOutput":
            shape = tuple(alloc.tensor_shape)
            np_dt = _mybir.dt.np(alloc.dtype)
            out_names.append(name)
            out_avals.append(jax.core.ShapedArray(shape, np_dt))
            zero_outs.append(np.zeros(shape, np_dt))
    n_params = len(in_names)
    all_in_names = list(in_names) + list(out_names)
    if partition_name is not None:
        all_in_names.append(partition_name)

    def _body(*args):
        operands = list(args)
        if partition_name is not None:
            operands.append(bass2jax.partition_id_tensor())
        outs = bass2jax._bass_exec_p.bind(
            *operands,
            out_avals=tuple(out_avals),
            in_names=tuple(all_in_names),
            out_names=tuple(out_names),
            lowering_input_output_aliases=(),
            sim_require_finite=True,
            sim_require_nnan=True,
            nc=nc,
        )
        return tuple(outs)

    devices = jax.devices()[:N_CORES]
    mesh = Mesh(np.asarray(devices), ("core",))
    in_specs = (PartitionSpec("core"),) * (n_params + len(out_names))
    out_specs = (PartitionSpec("core"),) * len(out_names)
    sharded = jax.jit(shard_map(_body, mesh=mesh, in_specs=in_specs,
                                out_specs=out_specs, check_rep=False),
                      keep_unused=True)

    concat_zeros = [np.zeros((N_CORES * z.shape[0], *z.shape[1:]), z.dtype)
                    for z in zero_outs]

    def run(in_maps, iters=1, time_list=None):
        import time as _time
        per_core = [[np.asarray(m[nm]) for nm in in_names] for m in in_maps]
        concat_in = [np.concatenate([per_core[c][i] for c in range(N_CORES)], axis=0)
                     for i in range(n_params)]
        dev_in = [jax.device_put(a) for a in concat_in]
        dev_zero = [jax.device_put(z) for z in concat_zeros]
        out_arrs = None
        for _ in range(max(1, iters)):
            t0 = _time.perf_counter()
            out_arrs = sharded(*dev_in, *dev_zero)
            jax.block_until_ready(out_arrs)
            if time_list is not None:
                time_list.append(_time.perf_counter() - t0)
        return [
            {nm: np.asarray(out_arrs[i]).reshape(N_CORES, *out_avals[i].shape)[c]
             for i, nm in enumerate(out_names)}
            for c in range(N_CORES)
        ]

    _RUNNER = run
    return run


def _host_tables(q_gamma, k_gamma):
    pos = np.arange(S, dtype=np.float32)
    inv = 1.0 / (ROPE_FREQ ** (np.arange(0, HEAD_DIM, 2, dtype=np.float32)
                               / HEAD_DIM))
    fr = pos[:, None] * inv[None, :]
    emb = np.concatenate([fr, fr], axis=-1)
    cos = np.cos(emb).astype(np.float32)
    sin = np.sin(emb).astype(np.float32)
    outs = []
    for gamma in (q_gamma, k_gamma):
        g = gamma.astype(np.float32)
        cos_g = cos * g[None, :]
        sin_eff = np.concatenate([-sin[:, :32] * g[None, 32:],
                                  sin[:, 32:] * g[None, :32]], axis=-1)
        outs += [cos_g, sin_eff]
    return outs  # cos_q, sin_q, cos_k, sin_k


def _make_in_maps(x, w_qkv, w_out, q_gamma, k_gamma):
    cos_q, sin_q, cos_k, sin_k = _host_tables(q_gamma, k_gamma)
    mask = (np.arange(128)[None, :] >= np.arange(128)[:, None]).astype(np.float16)
    ident = np.eye(128, dtype=np.float16)

    in_maps = []
    for core in range(N_CORES):
        b, g = core // 4, core % 4
        xT = np.ascontiguousarray(x[b].T).astype(np.float16)
        wq = w_qkv[512 * g:512 * (g + 1)]                      # 8 q heads
        wk = w_qkv[2048 + 128 * g:2048 + 128 * (g + 1)]        # 2 k heads
        wv = w_qkv[2560 + 128 * g:2560 + 128 * (g + 1)]        # 2 v heads
        wqkvT = np.ascontiguousarray(
            np.concatenate([wq, wk, wv], axis=0).T).astype(np.float16)
        woutT = np.ascontiguousarray(
            w_out[:, 512 * g:512 * (g + 1)].T).astype(np.float16)
        in_maps.append({
            "xT": xT, "wqkvT": wqkvT, "woutT": woutT,
            "cos_q": cos_q, "sin_q": sin_q, "cos_k": cos_k, "sin_k": sin_k,
            "mask_i": mask, "ident_i": ident,
        })
    return in_maps


def kernel(x, w_qkv, w_out, q_gamma, k_gamma):
    x = np.asarray(x)
    w_qkv = np.asarray(w_qkv)
    w_out = np.asarray(w_out)
    q_gamma = np.asarray(q_gamma)
    k_gamma = np.asarray(k_gamma)
    in_maps = _make_in_maps(x, w_qkv, w_out, q_gamma, k_gamma)
    results = _get_runner()(in_maps)
    parts = [results[c]["out"] for c in range(N_CORES)]
    out = np.empty((B, S, D), dtype=np.float32)
    for b in range(B):
        out[b] = parts[4 * b] + parts[4 * b + 1] + parts[4 * b + 2] + parts[4 * b + 3]
    return out


# revision 6
# speedup vs baseline: 39.3767x; 39.3767x over previous
"""Multi-head causal attention (GQA + QK-RMSNorm + RoPE) for Trainium2, 8 cores.

Sharding: 8 cores = 2 batches x 4 head-groups (tensor parallel over heads).
Each core handles one batch and 8 Q heads + 2 KV heads:
  - QKV projection for its head slice (fp16 matmuls, fp32 psum)
  - QK RMSNorm + RoPE (gamma folded into host-side cos/sin tables)
  - causal attention in k-major (transposed-scores) layout:
      sT[k, q] = kT.T @ qT ; p = exp(sT/8) ; ctx = pT.T @ [v | 1]
    (the ones column yields the softmax denominator for free)
  - output projection against its w_out column slice -> partial [S, D]
Host sums the 4 head-group partials per batch.
"""
import sys
import numpy as np
from contextlib import ExitStack

if '/opt/trn_rl_repo' not in sys.path:
    sys.path.insert(0, '/opt/trn_rl_repo')

import concourse.bacc as bacc
import concourse.tile as tile
import concourse.mybir as mybir
from concourse.bass_utils import run_bass_kernel_spmd

dt = mybir.dt
AF = mybir.ActivationFunctionType
AX = mybir.AxisListType
ALU = mybir.AluOpType

HEAD_DIM = 64
NUM_Q_HEADS = 32
NUM_KV_HEADS = 8
ROPE_FREQ = 10000.0
EPS = 1e-6

B, S, D = 2, 2048, 2048
QH = 8            # q heads per core
KVH = 2           # kv heads per core
N_CORES = 8
ST = S // 128      # 16 token tiles of 128
NJ = S // 512      # 4 big q blocks of 512

F16 = dt.float16
F32 = dt.float32


def _build():
    nc = bacc.Bacc("TRN2", target_bir_lowering=False, debug=False,
                   num_devices=N_CORES)

    xT = nc.dram_tensor("xT", [D, S], F16, kind="ExternalInput").ap()
    wqkvT = nc.dram_tensor("wqkvT", [D, (QH + 2 * KVH) * HEAD_DIM], F16,
                           kind="ExternalInput").ap()          # cols: 512 q | 128 k | 128 v
    woutT = nc.dram_tensor("woutT", [QH * HEAD_DIM, D], F16,
                           kind="ExternalInput").ap()
    cos_q = nc.dram_tensor("cos_q", [S, HEAD_DIM], F32, kind="ExternalInput").ap()
    sin_q = nc.dram_tensor("sin_q", [S, HEAD_DIM], F32, kind="ExternalInput").ap()
    cos_k = nc.dram_tensor("cos_k", [S, HEAD_DIM], F32, kind="ExternalInput").ap()
    sin_k = nc.dram_tensor("sin_k", [S, HEAD_DIM], F32, kind="ExternalInput").ap()
    mask_i = nc.dram_tensor("mask_i", [128, 128], F16, kind="ExternalInput").ap()
    ident_i = nc.dram_tensor("ident_i", [128, 128], F16, kind="ExternalInput").ap()
    out = nc.dram_tensor("out", [S, D], F32, kind="ExternalOutput").ap()

    with tile.TileContext(nc) as tc, ExitStack() as ctx:
        perm = ctx.enter_context(tc.tile_pool(name="perm", bufs=1))

        # ---- persistent tiles ----
        w_ch = [perm.tile([128, 768], F16, tag=f"w{dc}", name=f"w{dc}") for dc in range(16)]
        for dc in range(16):
            nc.sync.dma_start(w_ch[dc][:], wqkvT[128 * dc:128 * (dc + 1), :])
        wo_ch = [perm.tile([128, D], F16, tag=f"wo{p}", name=f"wo{p}") for p in range(4)]
        for p in range(4):
            nc.sync.dma_start(wo_ch[p][:], woutT[128 * p:128 * (p + 1), :])
        tabs = {}
        for nm, src in (("cq", cos_q), ("sq", sin_q), ("ck", cos_k), ("sk", sin_k)):
            t = perm.tile([128, ST, HEAD_DIM], F32, tag=f"tab{nm}", name=f"tab{nm}")
            nc.sync.dma_start(t[:], src.rearrange("(n p) d -> p n d", p=128))
            tabs[nm] = t
        tmask = perm.tile([128, 128], F16, tag="mask")
        nc.sync.dma_start(tmask[:], mask_i[:])
        tident = perm.tile([128, 128], F16, tag="ident")
        nc.sync.dma_start(tident[:], ident_i[:])
        teps = perm.tile([128, 1], F32, tag="eps")
        nc.vector.memset(teps[:], EPS)

        qT = [perm.tile([128, S], F16, tag=f"qT{p}", name=f"qT{p}") for p in range(4)]
        kT = [perm.tile([128, S], F16, tag=f"kT{g}", name=f"kT{g}") for g in range(KVH)]
        vext = [[perm.tile([128, HEAD_DIM + 1], F16, tag=f"v{g}_{i}", name=f"v{g}_{i}")
                 for i in range(ST)] for g in range(KVH)]
        for g in range(KVH):
            for i in range(ST):
                nc.vector.memset(vext[g][i][:, HEAD_DIM:HEAD_DIM + 1], 1.0)
        ctxT = [perm.tile([128, S], F16, tag=f"ctxT{p}", name=f"ctxT{p}") for p in range(4)]

        # ================= Phase 1: QKV + norm + rope + transposes ============
        with tc.tile_pool(name="p1sb", bufs=3) as p1sb, \
             tc.tile_pool(name="p1st", bufs=2) as p1st, \
             tc.tile_pool(name="psq", bufs=2, space="PSUM") as psq, \
             tc.tile_pool(name="pskv", bufs=2, space="PSUM") as pskv, \
             tc.tile_pool(name="pstp", bufs=2, space="PSUM") as pstp:
            for i2 in range(ST // 2):            # load x in 256-token chunks
                xt = p1sb.tile([128, 16, 256], F16, tag="xt")
                nc.sync.dma_start(
                    xt[:], xT[:, 256 * i2:256 * (i2 + 1)]
                    .rearrange("(n p) s -> p n s", p=128))
                for ii in range(2):
                    i = 2 * i2 + ii
                    pq = psq.tile([128, 512], F32, tag="pq")
                    pkv = pskv.tile([128, 256], F32, tag="pkv")
                    for dc in range(16):
                        xs = xt[:, dc, 128 * ii:128 * (ii + 1)]
                        nc.tensor.matmul(pq[:], xs, w_ch[dc][:, 0:512],
                                         start=(dc == 0), stop=(dc == 15))
                        nc.tensor.matmul(pkv[:], xs, w_ch[dc][:, 512:768],
                                         start=(dc == 0), stop=(dc == 15))
                    qsb = p1sb.tile([128, 640], F16, tag="qsb")
                    nc.vector.tensor_copy(qsb[:, 0:512], pq[:])
                    nc.vector.tensor_copy(qsb[:, 512:640], pkv[:, 0:128])
                    # v chunks straight out to vext tiles
                    nc.vector.tensor_copy(vext[0][i][:, 0:HEAD_DIM], pkv[:, 128:192])
                    nc.vector.tensor_copy(vext[1][i][:, 0:HEAD_DIM], pkv[:, 192:256])

                    # --- RMSNorm (per 64) + RoPE on q (8 heads) and k (2 heads)
                    sqt = p1sb.tile([128, 640], F16, tag="sqt")
                    nc.vector.tensor_mul(sqt[:], qsb[:], qsb[:])
                    ssum = p1st.tile([128, 10], F32, tag="ssum")
                    nc.vector.tensor_reduce(
                        ssum[:], sqt[:].rearrange("p (h d) -> p h d", h=10),
                        axis=AX.X, op=ALU.add)
                    std = p1st.tile([128, 10], F32, tag="std")
                    nc.scalar.activation(std[:], ssum[:], AF.Sqrt,
                                         bias=teps[:], scale=1.0 / HEAD_DIM)
                    rstd = p1st.tile([128, 10], F32, tag="rstd")
                    nc.vector.reciprocal(rstd[:], std[:])
                    qn = p1sb.tile([128, 640], F16, tag="qn")
                    nc.vector.tensor_mul(
                        qn[:].rearrange("p (h d) -> p h d", h=10),
                        qsb[:].rearrange("p (h d) -> p h d", h=10),
                        rstd[:].unsqueeze(2).broadcast_to([128, 10, HEAD_DIM]))
                    qr = p1sb.tile([128, 640], F16, tag="qr")
                    t2 = p1sb.tile([128, 640], F16, tag="t2")
                    for part, nh, c_t, s_t in (("q", QH, "cq", "sq"),
                                               ("k", KVH, "ck", "sk")):
                        off = 0 if part == "q" else 512
                        qn3 = qn[:, off:off + 64 * nh].rearrange(
                            "p (h d) -> p h d", h=nh)
                        qn4 = qn[:, off:off + 64 * nh].rearrange(
                            "p (h two x) -> p h two x", h=nh, two=2)
                        cosb = tabs[c_t][:, i, :].unsqueeze(1) \
                            .broadcast_to([128, nh, HEAD_DIM])
                        sin4 = tabs[s_t][:, i, :].unsqueeze(1) \
                            .broadcast_to([128, nh, HEAD_DIM]) \
                            .rearrange("p h (two x) -> p h two x", two=2)
                        t2v = t2[:, off:off + 64 * nh].rearrange(
                            "p (h two x) -> p h two x", h=nh, two=2)
                        nc.vector.tensor_mul(t2v[:, :, 0, :], qn4[:, :, 1, :],
                                             sin4[:, :, 0, :])
                        nc.vector.tensor_mul(t2v[:, :, 1, :], qn4[:, :, 0, :],
                                             sin4[:, :, 1, :])
                        # qr = qn*cos + t2  (two ops: mul into qr, then add)
                        qr3 = qr[:, off:off + 64 * nh].rearrange(
                            "p (h d) -> p h d", h=nh)
                        nc.vector.tensor_mul(qr3, qn3, cosb)
                    nc.vector.tensor_add(qr[:], qr[:], t2[:])

                    # --- transposes into [hd, S] pair layout
                    for p in range(4):
                        tp = pstp.tile([128, 128], F16, tag="tpq")
                        nc.tensor.transpose(tp[0:64, :], qr[:, 128 * p:128 * p + 64],
                                            tident[:])
                        nc.tensor.transpose(tp[64:128, :],
                                            qr[:, 128 * p + 64:128 * p + 128],
                                            tident[:])
                        nc.vector.tensor_copy(qT[p][:, 128 * i:128 * (i + 1)], tp[:])
                    for g in range(KVH):
                        tp = pstp.tile([128, 128], F16, tag="tpk")
                        nc.tensor.transpose(tp[0:64, :],
                                            qr[:, 512 + 64 * g:512 + 64 * (g + 1)],
                                            tident[:])
                        nc.tensor.transpose(tp[64:128, :],
                                            qr[:, 512 + 64 * g:512 + 64 * (g + 1)],
                                            tident[:])
                        nc.vector.tensor_copy(kT[g][:, 128 * i:128 * (i + 1)], tp[:])

        # ================= Phase 2: attention =================================
        with tc.tile_pool(name="pssT", bufs=2, space="PSUM") as pssT, \
             tc.tile_pool(name="psctx", bufs=1, space="PSUM") as psctx, \
             tc.tile_pool(name="pstp2", bufs=2, space="PSUM") as pstp2, \
             tc.tile_pool(name="ptpool", bufs=4) as ptpool, \
             tc.tile_pool(name="cnpool", bufs=2) as cnpool, \
             tc.tile_pool(name="rcpool", bufs=8) as rcpool:
            for J in range(NJ):
                ctxn = [cnpool.tile([128, 512], F16, tag=f"cn{jj}", name=f"cn{jj}")
                        for jj in range(4)]
                for h in range(QH):
                    g = h // 4          # kv head
                    pair = h // 2
                    half = 64 * (h % 2)
                    cps = [psctx.tile([128, HEAD_DIM + 1], F32, tag=f"ctx{jj}", name=f"ctx{jj}")
                           for jj in range(4)]
                    for c in range(4 * J + 4):
                        sT = pssT.tile([128, 512], F32, tag="sT")
                        nc.tensor.matmul(
                            sT[:], kT[g][half:half + 64, 128 * c:128 * (c + 1)],
                            qT[pair][half:half + 64, 512 * J:512 * (J + 1)],
                            start=True, stop=True)
                        jj0 = max(0, c - 4 * J)
                        pt = ptpool.tile([128, 512], F16, tag="pt")
                        nc.scalar.activation(pt[:, 128 * jj0:512],
                                             sT[:, 128 * jj0:512],
                                             AF.Exp, scale=0.125)
                        if c >= 4 * J:      # diagonal block: triangular mask
                            nc.vector.tensor_mul(
                                pt[:, 128 * jj0:128 * (jj0 + 1)],
                                pt[:, 128 * jj0:128 * (jj0 + 1)], tmask[:])
                        for jj in range(jj0, 4):
                            nc.tensor.matmul(
                                cps[jj][:], pt[:, 128 * jj:128 * (jj + 1)],
                                vext[g][c][:],
                                start=(c == 0), stop=(c == 4 * J + jj))
                    for jj in range(4):
                        rc = rcpool.tile([128, 1], F32, tag="rc")
                        nc.vector.reciprocal(
                            rc[:], cps[jj][:, HEAD_DIM:HEAD_DIM + 1])
                        nc.vector.tensor_scalar_mul(
                            ctxn[jj][:, 64 * h:64 * (h + 1)],
                            cps[jj][:, 0:HEAD_DIM], rc[:])
                for jj in range(4):
                    for p in range(4):
                        tp = pstp2.tile([128, 128], F16, tag="tp2")
                        nc.tensor.transpose(
                            tp[0:64, :], ctxn[jj][:, 128 * p:128 * p + 64],
                            tident[:])
                        nc.tensor.transpose(
                            tp[64:128, :], ctxn[jj][:, 128 * p + 64:128 * (p + 1)],
                            tident[:])
                        nc.vector.tensor_copy(
                            ctxT[p][:, 512 * J + 128 * jj:512 * J + 128 * (jj + 1)],
                            tp[:])

        # ================= Phase 3: output projection =========================
        with tc.tile_pool(name="psout", bufs=4, space="PSUM") as psout, \
             tc.tile_pool(name="osb", bufs=4) as osb:
            for i in range(ST):
                for dch in range(4):
                    po = psout.tile([128, 512], F32, tag="po")
                    for p in range(4):
                        nc.tensor.matmul(
                            po[:], ctxT[p][:, 128 * i:128 * (i + 1)],
                            wo_ch[p][:, 512 * dch:512 * (dch + 1)],
                            start=(p == 0), stop=(p == 3))
                    ob = osb.tile([128, 512], F32, tag="ob")
                    nc.scalar.copy(ob[:], po[:])
                    nc.sync.dma_start(
                        out[128 * i:128 * (i + 1), 512 * dch:512 * (dch + 1)],
                        ob[:])

    nc.compile()
    return nc


_NC = None


def _get_nc():
    global _NC
    if _NC is None:
        _NC = _build()
    return _NC


_RUNNER = None


def _get_runner():
    """Build (once) a jitted 8-core sharded callable around the bass program.

    Slim replica of bass2jax.run_bass_via_pjrt's multi-core path, kept
    reusable so repeated invocations skip retracing/recompilation.
    """
    global _RUNNER
    if _RUNNER is not None:
        return _RUNNER
    import jax
    from jax.sharding import Mesh, PartitionSpec
    from jax.experimental.shard_map import shard_map
    from concourse import bass2jax
    from concourse import mybir as _mybir

    nc = _get_nc()
    bass2jax.install_neuronx_cc_hook()

    partition_name = nc.partition_id_tensor.name if nc.partition_id_tensor else None
    in_names, out_names, out_avals, zero_outs = [], [], [], []
    for alloc in nc.m.functions[0].allocations:
        if not isinstance(alloc, _mybir.MemoryLocationSet):
            continue
        name = alloc.memorylocations[0].name
        if alloc.kind == "ExternalInput":
            if name != partition_name:
                in_names.append(name)
        elif alloc.kind == "ExternalOutput":
            shape = tuple(alloc.tensor_shape)
            np_dt = _mybir.dt.np(alloc.dtype)
            out_names.append(name)
            out_avals.append(jax.core.ShapedArray(shape, np_dt))
            zero_outs.append(np.zeros(shape, np_dt))
    n_params = len(in_names)
    all_in_names = list(in_names) + list(out_names)
    if partition_name is not None:
        all_in_names.append(partition_name)

    def _body(*args):
        operands = list(args)
        if partition_name is not None:
            operands.append(bass2jax.partition_id_tensor())
        outs = bass2jax._bass_exec_p.bind(
            *operands,
            out_avals=tuple(out_avals),
            in_names=tuple(all_in_names),
            out_names=tuple(out_names),
            lowering_input_output_aliases=(),
            sim_require_finite=True,
            sim_require_nnan=True,
            nc=nc,
        )
        return tuple(outs)

    devices = jax.devices()[:N_CORES]
    mesh = Mesh(np.asarray(devices), ("core",))
    in_specs = (PartitionSpec("core"),) * (n_params + len(out_names))
    out_specs = (PartitionSpec("core"),) * len(out_names)
    sharded = jax.jit(shard_map(_body, mesh=mesh, in_specs=in_specs,
                                out_specs=out_specs, check_rep=False),
                      keep_unused=True)

    concat_zeros = [np.zeros((N_CORES * z.shape[0], *z.shape[1:]), z.dtype)
                    for z in zero_outs]

    def run(in_maps, iters=1, time_list=None):
        import time as _time
        from jax.sharding import NamedSharding
        shard = NamedSharding(mesh, PartitionSpec("core"))
        per_core = [[np.asarray(m[nm]) for nm in in_names] for m in in_maps]
        concat_in = [np.concatenate([per_core[c][i] for c in range(N_CORES)], axis=0)
                     for i in range(n_params)]
        dev_in = [jax.device_put(a, shard) for a in concat_in]
        dev_zero = [jax.device_put(z, shard) for z in concat_zeros]
        out_arrs = None
        for _ in range(max(1, iters)):
            t0 = _time.perf_counter()
            out_arrs = sharded(*dev_in, *dev_zero)
            jax.block_until_ready(out_arrs)
            if time_list is not None:
                time_list.append(_time.perf_counter() - t0)
        return [
            {nm: np.asarray(out_arrs[i]).reshape(N_CORES, *out_avals[i].shape)[c]
             for i, nm in enumerate(out_names)}
            for c in range(N_CORES)
        ]

    _RUNNER = run
    return run


def _host_tables(q_gamma, k_gamma):
    pos = np.arange(S, dtype=np.float32)
    inv = 1.0 / (ROPE_FREQ ** (np.arange(0, HEAD_DIM, 2, dtype=np.float32)
                               / HEAD_DIM))
    fr = pos[:, None] * inv[None, :]
    emb = np.concatenate([fr, fr], axis=-1)
    cos = np.cos(emb).astype(np.float32)
    sin = np.sin(emb).astype(np.float32)
    outs = []
    for gamma in (q_gamma, k_gamma):
        g = gamma.astype(np.float32)
        cos_g = cos * g[None, :]
        sin_eff = np.concatenate([-sin[:, :32] * g[None, 32:],
                                  sin[:, 32:] * g[None, :32]], axis=-1)
        outs += [cos_g, sin_eff]
    return outs  # cos_q, sin_q, cos_k, sin_k


def _make_in_maps(x, w_qkv, w_out, q_gamma, k_gamma):
    cos_q, sin_q, cos_k, sin_k = _host_tables(q_gamma, k_gamma)
    mask = (np.arange(128)[None, :] >= np.arange(128)[:, None]).astype(np.float16)
    ident = np.eye(128, dtype=np.float16)

    in_maps = []
    for core in range(N_CORES):
        b, g = core // 4, core % 4
        xT = np.ascontiguousarray(x[b].T).astype(np.float16)
        wq = w_qkv[512 * g:512 * (g + 1)]                      # 8 q heads
        wk = w_qkv[2048 + 128 * g:2048 + 128 * (g + 1)]        # 2 k heads
        wv = w_qkv[2560 + 128 * g:2560 + 128 * (g + 1)]        # 2 v heads
        wqkvT = np.ascontiguousarray(
            np.concatenate([wq, wk, wv], axis=0).T).astype(np.float16)
        woutT = np.ascontiguousarray(
            w_out[:, 512 * g:512 * (g + 1)].T).astype(np.float16)
        in_maps.append({
            "xT": xT, "wqkvT": wqkvT, "woutT": woutT,
            "cos_q": cos_q, "sin_q": sin_q, "cos_k": cos_k, "sin_k": sin_k,
            "mask_i": mask, "ident_i": ident,
        })
    return in_maps


def kernel(x, w_qkv, w_out, q_gamma, k_gamma):
    x = np.asarray(x)
    w_qkv = np.asarray(w_qkv)
    w_out = np.asarray(w_out)
    q_gamma = np.asarray(q_gamma)
    k_gamma = np.asarray(k_gamma)
    in_maps = _make_in_maps(x, w_qkv, w_out, q_gamma, k_gamma)
    results = _get_runner()(in_maps)
    parts = [results[c]["out"] for c in range(N_CORES)]
    out = np.empty((B, S, D), dtype=np.float32)
    for b in range(B):
        out[b] = parts[4 * b] + parts[4 * b + 1] + parts[4 * b + 2] + parts[4 * b + 3]
    return out
